# revision 19
# baseline (speedup 1.0000x reference)
"""Trainium2 Bass kernel for nn_ChannelShuffle (topk_masking).

Reference computation (per sample i of N=80, c=2048 channels, hw=256):
  scores = s_ca[i]                       # [c]
  topk_idx = top_k(scores, S=512)        # sorted desc, stable ties
  j = (i + 1 + partner[i]) % N
  blend[k] = 0.7*x[i, topk_idx[k]] + 0.3*x[j, rand_index[i, k]]
  aug = x[i] with channels topk_idx[k] <- blend[k]
  out[orig slot] = x[i] * scores ; out[aug slot] = aug * scores
  slots: g=way*16+t -> orig row way*32+t, aug row way*32+16+t (way=g//16)

Strategy: data-parallel over the batch dim, 10 samples per core (8 cores).
Host does index-only prep (argsort topk, partner mapping, gather/scatter
index streams, scale vectors); the device does all tensor math per sample:
  y   = x * s                  -> orig slot        (scalar engine)
  aug = x * A                  -> aug slot          (vector engine)
        where A = 0.7*s on topk channels else s
  xq  = dma_gather(x_part, rand rows) * (0.3*s_topk)   (rank space)
  one dma_scatter_add per sample accumulates xq onto the aug slot's topk
  rows (512 rows per instruction), ordered after the dense write by
  Tile's DRAM dependency tracking (verified on HW).
Alternative builders kept for reference: _build (indirect scatter-add or
TensorE one-hot merge via pe_merge=True) and _build_plain (overwrite
scatter with lookahead gathers).
"""

import numpy as np

# problem constants (hardcoded per harness contract)
N = 80          # batch
C = 2048        # channels
E = 256         # h*w = 16*16
S = 512         # shuffle_num
NCORES = 8
NLOC = N // NCORES          # samples per core
P = 128                     # partitions
CH = C // P                 # 16 free-dim chunks per sample; ch = p*CH + chunk
NRK = S // P                # 4 rank chunks; rank r = n*128 + p
SW = S // 16                # 32 idx stream cols for dma_gather

_CACHE = {}


def _build(n_loc=NLOC, reps=1, bufs=3, pe_merge=False):
    import concourse.bacc as bacc
    import concourse.tile as tile
    from concourse import bass, mybir

    nc = bacc.Bacc("TRN2", target_bir_lowering=False, debug=False,
                   num_devices=NCORES)

    x_own = nc.dram_tensor("x_own", [n_loc * C, E], mybir.dt.float32,
                           kind="ExternalInput")
    x_part = nc.dram_tensor("x_part", [n_loc * C, E], mybir.dt.float32,
                            kind="ExternalInput")
    # sscl cols: 0:CH = s (ch = p*CH+c); CH:2*CH = A (0.7*s on topk else s);
    # 2*CH:2*CH+NRK = 0.3*s_topk at rank slot (p, n)
    sscl = nc.dram_tensor("sscl", [n_loc, P, 2 * CH + 3 * NRK],
                          mybir.dt.float32, kind="ExternalInput")
    # gidx: int16 dma_gather stream (16-wrapped, core-replicated) of partner
    # rows in x_part
    gidx = nc.dram_tensor("gidx", [n_loc, P, 2 * SW], mybir.dt.int16,
                          kind="ExternalInput")
    # oidx: scatter dest rows (C + topk_idx) at rank slot (p, n)
    oidx = nc.dram_tensor("oidx", [n_loc, P, NRK], mybir.dt.int32,
                          kind="ExternalInput")
    outs = [
        nc.dram_tensor(f"out{i}", [2 * C, E], mybir.dt.float32,
                       kind="ExternalOutput")
        for i in range(n_loc)
    ]

    FREE = CH * E  # 4096 f32 per partition

    big_bufs = min(bufs, 2) if pe_merge else bufs
    with tile.TileContext(nc) as tc:
        with (
            tc.tile_pool(name="xp", bufs=big_bufs) as xpool,
            tc.tile_pool(name="yp", bufs=big_bufs) as ypool,
            tc.tile_pool(name="ap", bufs=big_bufs) as apool,
            tc.tile_pool(name="gp", bufs=bufs) as gpool,
            tc.tile_pool(name="sp", bufs=bufs) as spool,
            tc.tile_pool(name="scp", bufs=2) as scpool,
            tc.tile_pool(name="pp", bufs=4, space="PSUM") as ppool,
            tc.tile_pool(name="cp", bufs=1) as cpool,
        ):
            if pe_merge:
                # per-chunk channel iotas: iota_cI[p, f] = f*CH + cI (exact in
                # f32) — matches M2 channel layout ch = p*CH + cI per chunk
                iota_f = cpool.tile([P, CH * P], mybir.dt.float32, tag="iof")
                for cI in range(CH):
                    nc.gpsimd.iota(
                        iota_f[:, cI * P:(cI + 1) * P], [[CH, P]], base=cI,
                        channel_multiplier=0,
                        allow_small_or_imprecise_dtypes=True)

            for i in [i for _ in range(reps) for i in range(n_loc)]:
                x_sb = xpool.tile([P, FREE], mybir.dt.float32)
                nc.sync.dma_start(
                    x_sb[:],
                    x_own[i * C:(i + 1) * C].rearrange("(p c) e -> p (c e)", p=P),
                )
                sscl_sb = spool.tile([P, 2 * CH + 3 * NRK], mybir.dt.float32)
                nc.sync.dma_start(sscl_sb[:], sscl[i])
                gidx_sb = spool.tile([P, 2 * SW], mybir.dt.int16, tag="gidx")
                nc.sync.dma_start(gidx_sb[:], gidx[i])
                if not pe_merge:
                    oidx_sb = spool.tile([P, NRK], mybir.dt.int32, tag="oidx")
                    nc.sync.dma_start(oidx_sb[:], oidx[i])

                # partner rows, rank space: slot (p, n) = rank n*128+p
                xq_sb = gpool.tile([P, NRK * E], mybir.dt.float32)
                nc.gpsimd.dma_gather(
                    out_ap=xq_sb[:].rearrange("p (n e) -> p n e", e=E),
                    in_ap=x_part[:],
                    idxs_ap=gidx_sb[:, SW:2 * SW],
                    num_idxs=S,
                    num_idxs_reg=S,
                    elem_size=E,
                )
                # xq *= 0.3*s_topk (per rank slot)
                for n in range(NRK):
                    nc.vector.tensor_scalar_mul(
                        xq_sb[:, n * E:(n + 1) * E],
                        xq_sb[:, n * E:(n + 1) * E],
                        sscl_sb[:, 2 * CH + n:2 * CH + n + 1],
                    )

                # y = x*s (scalar engine)
                y_sb = ypool.tile([P, FREE], mybir.dt.float32)
                for cI in range(CH):
                    nc.scalar.activation(
                        y_sb[:, cI * E:(cI + 1) * E],
                        x_sb[:, cI * E:(cI + 1) * E],
                        mybir.ActivationFunctionType.Copy,
                        scale=sscl_sb[:, cI:cI + 1],
                    )

                a_sb = apool.tile([P, FREE], mybir.dt.float32)
                if pe_merge:
                    # one-hot selection: Sc[(n,cI)][k=p_rank, m] =
                    #   (topk[n*128+k] == m*CH + cI)  -> psum partition m gets
                    # channel m*CH+cI, matching a_sb chunk cI's layout
                    sc_sb = scpool.tile([P, NRK * C], mybir.dt.float32)
                    for n in range(NRK):
                        for cI in range(CH):
                            off = (n * CH + cI) * P
                            nc.vector.tensor_scalar(
                                sc_sb[:, off:off + P],
                                iota_f[:, cI * P:(cI + 1) * P],
                                sscl_sb[:, 2 * CH + NRK + n:
                                        2 * CH + NRK + n + 1],
                                None, op0=mybir.AluOpType.is_equal,
                            )
                    # delta[ch_chunk] = sum_n Sc_n[:, chunk]^T @ xq_n
                    for cI in range(CH):
                        ps = ppool.tile([P, E], mybir.dt.float32, space="PSUM")
                        for n in range(NRK):
                            off = (n * CH + cI) * P
                            nc.tensor.matmul(
                                ps[:],
                                sc_sb[:, off:off + P],
                                xq_sb[:, n * E:(n + 1) * E],
                                start=(n == 0),
                                stop=(n == NRK - 1),
                            )
                        # aug = x*A + delta
                        nc.vector.scalar_tensor_tensor(
                            a_sb[:, cI * E:(cI + 1) * E],
                            x_sb[:, cI * E:(cI + 1) * E],
                            sscl_sb[:, CH + cI:CH + cI + 1],
                            ps[:],
                            op0=mybir.AluOpType.mult,
                            op1=mybir.AluOpType.add,
                        )
                else:
                    for cI in range(CH):
                        nc.vector.tensor_scalar_mul(
                            a_sb[:, cI * E:(cI + 1) * E],
                            x_sb[:, cI * E:(cI + 1) * E],
                            sscl_sb[:, CH + cI:CH + cI + 1],
                        )

                nc.sync.dma_start(
                    outs[i][0:C].rearrange("(p c) e -> p (c e)", p=P), y_sb[:]
                )
                nc.sync.dma_start(
                    outs[i][C:2 * C].rearrange("(p c) e -> p (c e)", p=P), a_sb[:]
                )
                if not pe_merge:
                    # scatter-ADD blend remainder over the aug slot's topk rows
                    for n in range(NRK):
                        nc.gpsimd.indirect_dma_start(
                            out=outs[i][:],
                            out_offset=bass.IndirectOffsetOnAxis(
                                ap=oidx_sb[:, n:n + 1], axis=0
                            ),
                            in_=xq_sb[:, n * E:(n + 1) * E],
                            in_offset=None,
                            bounds_check=2 * C - 1,
                            oob_is_err=False,
                            compute_op=mybir.AluOpType.add,
                        )

    nc.compile()
    return nc


def _build_plain(n_loc=NLOC, reps=1, bufs=5, spread=False):
    """Plain-scatter design: y = x*s written to both slots, full blend
    (0.7*s*x_topk + 0.3*s*x_part) overwrites the aug slot's topk rows.
    y computed in place (frees SBUF for deeper buffering); sample i+1's
    gathers are issued before sample i's scatters so scatters never block
    gathers at the head of the Pool queue."""
    import concourse.bacc as bacc
    import concourse.tile as tile
    from concourse import bass, mybir

    nc = bacc.Bacc("TRN2", target_bir_lowering=False, debug=False,
                   num_devices=NCORES)
    x_own = nc.dram_tensor("x_own", [n_loc * C, E], mybir.dt.float32,
                           kind="ExternalInput")
    x_part = nc.dram_tensor("x_part", [n_loc * C, E], mybir.dt.float32,
                            kind="ExternalInput")
    sscl = nc.dram_tensor("sscl", [n_loc, P, 2 * CH + 3 * NRK],
                          mybir.dt.float32, kind="ExternalInput")
    gidx = nc.dram_tensor("gidx", [n_loc, P, 2 * SW], mybir.dt.int16,
                          kind="ExternalInput")
    oidx = nc.dram_tensor("oidx", [n_loc, P, NRK], mybir.dt.int32,
                          kind="ExternalInput")
    outs = [nc.dram_tensor(f"out{i}", [2 * C, E], mybir.dt.float32,
                           kind="ExternalOutput") for i in range(n_loc)]
    FREE = CH * E

    seq = [i for _ in range(reps) for i in range(n_loc)]
    with tile.TileContext(nc) as tc:
        with (
            tc.tile_pool(name="xp", bufs=bufs) as xpool,
            tc.tile_pool(name="gp", bufs=min(4, max(3, bufs - 1))) as gpool,
            tc.tile_pool(name="sp", bufs=min(4, max(3, bufs - 1))) as spool,
        ):
            def issue_gathers(i):
                sscl_sb = spool.tile([P, 2 * CH + 3 * NRK], mybir.dt.float32,
                                     tag="sscl")
                nc.sync.dma_start(sscl_sb[:], sscl[i])
                gidx_sb = spool.tile([P, 2 * SW], mybir.dt.int16, tag="gidx")
                nc.sync.dma_start(gidx_sb[:], gidx[i])
                oidx_sb = spool.tile([P, NRK], mybir.dt.int32, tag="oidx")
                nc.sync.dma_start(oidx_sb[:], oidx[i])
                xt_sb = gpool.tile([P, NRK * E], mybir.dt.float32, tag="xt")
                nc.gpsimd.dma_gather(
                    out_ap=xt_sb[:].rearrange("p (n e) -> p n e", e=E),
                    in_ap=x_own[:], idxs_ap=gidx_sb[:, 0:SW],
                    num_idxs=S, num_idxs_reg=S, elem_size=E)
                xq_sb = gpool.tile([P, NRK * E], mybir.dt.float32, tag="xq")
                nc.gpsimd.dma_gather(
                    out_ap=xq_sb[:].rearrange("p (n e) -> p n e", e=E),
                    in_ap=x_part[:], idxs_ap=gidx_sb[:, SW:2 * SW],
                    num_idxs=S, num_idxs_reg=S, elem_size=E)
                return sscl_sb, oidx_sb, xt_sb, xq_sb

            pend = issue_gathers(seq[0])
            for k, i in enumerate(seq):
                sscl_sb, oidx_sb, xt_sb, xq_sb = pend
                x_sb = xpool.tile([P, FREE], mybir.dt.float32)
                (nc.gpsimd if spread else nc.sync).dma_start(
                    x_sb[:],
                    x_own[i * C:(i + 1) * C].rearrange("(p c) e -> p (c e)", p=P))
                # next sample's gathers ahead of this sample's scatters
                if k + 1 < len(seq):
                    nxt = issue_gathers(seq[k + 1])
                # blend = 0.7*s_k*xt + 0.3*s_k*xq  (vector engine, in place)
                for n in range(NRK):
                    ts = xt_sb[:, n * E:(n + 1) * E]
                    qs = xq_sb[:, n * E:(n + 1) * E]
                    nc.vector.tensor_scalar_mul(
                        ts, ts, sscl_sb[:, 2 * CH + 2 * NRK + n:
                                        2 * CH + 2 * NRK + n + 1])
                    nc.vector.scalar_tensor_tensor(
                        ts, qs, sscl_sb[:, 2 * CH + n:2 * CH + n + 1], ts,
                        op0=mybir.AluOpType.mult, op1=mybir.AluOpType.add)
                # y = x*s in place (scalar engine)
                for cI in range(CH):
                    sl = slice(cI * E, (cI + 1) * E)
                    nc.scalar.activation(
                        x_sb[:, sl], x_sb[:, sl],
                        mybir.ActivationFunctionType.Copy,
                        scale=sscl_sb[:, cI:cI + 1])
                nc.sync.dma_start(
                    outs[i][0:C].rearrange("(p c) e -> p (c e)", p=P), x_sb[:])
                nc.scalar.dma_start(
                    outs[i][C:2 * C].rearrange("(p c) e -> p (c e)", p=P),
                    x_sb[:])
                # overwrite the aug slot's topk rows with the blend
                for n in range(NRK):
                    nc.gpsimd.indirect_dma_start(
                        out=outs[i][:],
                        out_offset=bass.IndirectOffsetOnAxis(
                            ap=oidx_sb[:, n:n + 1], axis=0),
                        in_=xt_sb[:, n * E:(n + 1) * E],
                        in_offset=None, bounds_check=2 * C - 1,
                        oob_is_err=False)
                if k + 1 < len(seq):
                    pend = nxt
    nc.compile()
    return nc


def _build_add2(n_loc=NLOC, reps=1, bufs=4):
    """Scatter-add design with the custom dma_scatter_add op: aug base =
    x*A (A = 0.7*s on topk else s) written densely; ONE dma_scatter_add
    per sample accumulates 0.3*s_topk*x_part onto the aug slot's topk rows
    (512 rows per instruction vs 4x128 for indirect DMA)."""
    import concourse.bacc as bacc
    import concourse.tile as tile
    from concourse import mybir

    nc = bacc.Bacc("TRN2", target_bir_lowering=False, debug=False,
                   num_devices=NCORES)
    x_own = nc.dram_tensor("x_own", [n_loc * C, E], mybir.dt.float32,
                           kind="ExternalInput")
    x_part = nc.dram_tensor("x_part", [n_loc * C, E], mybir.dt.float32,
                            kind="ExternalInput")
    sscl = nc.dram_tensor("sscl", [n_loc, P, 2 * CH + 3 * NRK],
                          mybir.dt.float32, kind="ExternalInput")
    gidx = nc.dram_tensor("gidx", [n_loc, P, 2 * SW], mybir.dt.int16,
                          kind="ExternalInput")
    oidx16 = nc.dram_tensor("oidx16", [n_loc, P, SW], mybir.dt.int16,
                            kind="ExternalInput")
    outs = [nc.dram_tensor(f"out{i}", [2 * C, E], mybir.dt.float32,
                           kind="ExternalOutput") for i in range(n_loc)]
    FREE = CH * E

    seq = [i for _ in range(reps) for i in range(n_loc)]
    with tile.TileContext(nc) as tc:
        with (
            tc.tile_pool(name="xp", bufs=bufs) as xpool,
            tc.tile_pool(name="ap2", bufs=bufs) as apool,
            tc.tile_pool(name="gp", bufs=min(4, bufs)) as gpool,
            tc.tile_pool(name="sp", bufs=min(4, bufs)) as spool,
        ):
            def issue_gathers(i):
                sscl_sb = spool.tile([P, 2 * CH + 3 * NRK], mybir.dt.float32,
                                     tag="sscl")
                nc.sync.dma_start(sscl_sb[:], sscl[i])
                gidx_sb = spool.tile([P, 2 * SW], mybir.dt.int16, tag="gidx")
                nc.sync.dma_start(gidx_sb[:], gidx[i])
                oidx_sb = spool.tile([P, SW], mybir.dt.int16, tag="oidx")
                nc.sync.dma_start(oidx_sb[:], oidx16[i])
                xq_sb = gpool.tile([P, NRK * E], mybir.dt.float32, tag="xq")
                nc.gpsimd.dma_gather(
                    out_ap=xq_sb[:].rearrange("p (n e) -> p n e", e=E),
                    in_ap=x_part[:], idxs_ap=gidx_sb[:, SW:2 * SW],
                    num_idxs=S, num_idxs_reg=S, elem_size=E)
                return sscl_sb, oidx_sb, xq_sb

            pend = issue_gathers(seq[0])
            for k, i in enumerate(seq):
                sscl_sb, oidx_sb, xq_sb = pend
                x_sb = xpool.tile([P, FREE], mybir.dt.float32)
                nc.sync.dma_start(
                    x_sb[:],
                    x_own[i * C:(i + 1) * C].rearrange("(p c) e -> p (c e)", p=P))
                if k + 1 < len(seq):
                    nxt = issue_gathers(seq[k + 1])
                # xq *= 0.3*s_topk (rank slots)
                for n in range(NRK):
                    nc.vector.tensor_scalar_mul(
                        xq_sb[:, n * E:(n + 1) * E],
                        xq_sb[:, n * E:(n + 1) * E],
                        sscl_sb[:, 2 * CH + n:2 * CH + n + 1])
                # a = x*A (vector); y = x*s in place (scalar)
                a_sb = apool.tile([P, FREE], mybir.dt.float32)
                for cI in range(CH):
                    sl = slice(cI * E, (cI + 1) * E)
                    nc.vector.tensor_scalar_mul(
                        a_sb[:, sl], x_sb[:, sl],
                        sscl_sb[:, CH + cI:CH + cI + 1])
                    nc.scalar.activation(
                        x_sb[:, sl], x_sb[:, sl],
                        mybir.ActivationFunctionType.Copy,
                        scale=sscl_sb[:, cI:cI + 1])
                nc.sync.dma_start(
                    outs[i][0:C].rearrange("(p c) e -> p (c e)", p=P), x_sb[:])
                nc.scalar.dma_start(
                    outs[i][C:2 * C].rearrange("(p c) e -> p (c e)", p=P),
                    a_sb[:])
                # one scatter-add of all 512 blend rows onto the aug slot
                nc.gpsimd.dma_scatter_add(
                    out_ap=outs[i][:],
                    in_ap=xq_sb[:].rearrange("p (n e) -> p n e", e=E),
                    idxs_ap=oidx_sb[:],
                    num_idxs=S, num_idxs_reg=S, elem_size=E)
                if k + 1 < len(seq):
                    pend = nxt
    nc.compile()
    return nc


def _get_nc(n_loc=NLOC, reps=1, mode="add2", bufs=None, spread=False):
    key = (n_loc, reps, mode, bufs, spread)
    if key not in _CACHE:
        if mode == "plain":
            _CACHE[key] = _build_plain(n_loc, reps, bufs or 5, spread)
        elif mode == "add2":
            _CACHE[key] = _build_add2(n_loc, reps, bufs or 4)
        elif mode == "pe":
            _CACHE[key] = _build(n_loc, reps, bufs or 3, pe_merge=True)
        else:
            _CACHE[key] = _build(n_loc, reps, bufs or 3, pe_merge=False)
    return _CACHE[key]


def _wrap16(stream):
    """[S] stream -> [P, S//16] int16 tile (16-wrapped, replicated per core)."""
    t = stream.reshape(S // 16, 16).T.astype(np.int16)     # [16, S//16]
    return np.tile(t, (8, 1))                              # [128, S//16]


def _prep(x, s_ca, rand_index, partner):
    """Host-side index/scale prep. Returns per-core input maps."""
    scores = np.asarray(s_ca, np.float32).reshape(N, C)
    x = np.ascontiguousarray(np.asarray(x, np.float32).reshape(N, C, E))
    rand_index = np.asarray(rand_index).astype(np.int64).reshape(N, S)
    partner = np.asarray(partner).astype(np.int64).reshape(N)

    # top-k (stable desc sort == jax.lax.top_k tie semantics)
    order = np.argsort(-scores, axis=1, kind="stable")
    topk = order[:, :S]                                    # [N, S]
    j = (np.arange(N) + 1 + partner) % N                   # partner sample

    rows = np.arange(N)
    i_loc = rows % NLOC
    s_topk = np.take_along_axis(scores, topk, axis=1)      # [N, S]

    a_v = scores.copy()
    np.put_along_axis(a_v, topk, np.float32(0.7) * s_topk, axis=1)

    sscl = np.concatenate([
        scores.reshape(N, P, CH),
        a_v.reshape(N, P, CH),
        (np.float32(0.3) * s_topk).reshape(N, NRK, P).transpose(0, 2, 1),
        topk.astype(np.float32).reshape(N, NRK, P).transpose(0, 2, 1),
        (np.float32(0.7) * s_topk).reshape(N, NRK, P).transpose(0, 2, 1),
    ], axis=2).astype(np.float32)                        # [N, P, 2*CH+3*NRK]

    # partner gather stream (rank order): rows in x_part flat tensor
    st_topk = (i_loc[:, None] * C + topk).astype(np.int64)         # [N, S]
    st_part = (i_loc[:, None] * C + rand_index).astype(np.int64)   # [N, S]
    gidx = np.empty((N, P, 2 * SW), np.int16)
    for g in range(N):
        gidx[g, :, :SW] = _wrap16(st_topk[g])
        gidx[g, :, SW:] = _wrap16(st_part[g])

    # scatter rows at rank slot (p, n): C + topk_idx[g, n*128+p]
    oidx = (C + topk).reshape(N, NRK, P).transpose(0, 2, 1).astype(np.int32)
    oidx16 = np.empty((N, P, SW), np.int16)
    for g in range(N):
        oidx16[g] = _wrap16(C + topk[g])

    in_maps = []
    for k in range(NCORES):
        sl = slice(k * NLOC, (k + 1) * NLOC)
        in_maps.append({
            "x_own": x[sl].reshape(NLOC * C, E),
            "x_part": np.ascontiguousarray(x[j[sl]]).reshape(NLOC * C, E),
            "sscl": np.ascontiguousarray(sscl[sl]),
            "gidx": np.ascontiguousarray(gidx[sl]),
            "oidx": np.ascontiguousarray(oidx[sl]),
            "oidx16": np.ascontiguousarray(oidx16[sl]),
        })
    return in_maps


def _assemble(results):
    """Map per-core out{i} [2C, E] tensors into the full [2N, C, 16, 16]."""
    full = np.empty((2 * N, C, 16, 16), np.float32)
    for k in range(NCORES):
        for il in range(NLOC):
            oc = results[k][f"out{il}"].reshape(2, C, 16, 16)
            g = k * NLOC + il
            way, t = g // 16, g % 16
            full[way * 32 + t] = oc[0]
            full[way * 32 + 16 + t] = oc[1]
    return full


def _filter_inmaps(nc, in_maps):
    from concourse import mybir
    names = set()
    for alloc in nc.m.functions[0].allocations:
        if (isinstance(alloc, mybir.MemoryLocationSet)
                and alloc.kind == "ExternalInput"):
            names.add(alloc.memorylocations[0].name)
    return [{k: v for k, v in m.items() if k in names} for m in in_maps]


def kernel(x, s_ca, rand_index, partner, shuffle_num, _trace=False):
    from concourse import bass_utils

    assert int(shuffle_num) == S
    nc = _get_nc()
    in_maps = _filter_inmaps(nc, _prep(x, s_ca, rand_index, partner))
    res = bass_utils.run_bass_kernel_spmd(
        nc, in_maps, core_ids=list(range(NCORES)), trace=_trace
    )
    out = _assemble(res.results)
    if _trace:
        return out, res
    return out
